# revision 52
# baseline (speedup 1.0000x reference)
"""Trainium2 Bass kernel for nn_CombNetHE — fp8 DoubleRow version.

Strategy vs the bf16 baseline:
  - Layer 1 runs in fp8 (e4m3) with MatmulPerfMode.DoubleRow: each matmul
    contracts 2 k-chunks (256 deep) at 0.5 cycles/row -> 4x bf16 throughput.
  - Quantization error is handled per net:
      net o (feeds comp_max_tau + the (1-cond) branch): 1 fp8 pass + the
        "G-trick": z2 += 0.5*(xhat@(G-Ghat) + xl@G) accumulated straight into
        the layer-2 PSUM via tiny [1024,10] matmuls with host-precomputed
        G = W1@W2 matrices. Optimal-shrinkage correction of the relu-masked
        pre-activation error (rho^2 = 1/2 -> sqrt(2) error reduction, ~free).
      net f (dominates the output since cond ~= 1 for most rows): 3 fp8
        passes (x8@w8 + x8@r8 + xl8@w8, residual-compensated to ~bf16
        accuracy).
  - Scale folding: ship 16*x, 64*W1 (and residuals at the same scales) so all
    passes accumulate at scale 1024; b1 is shipped *1024 and W2 /1024, so no
    on-device descaling is needed anywhere.
  - relu+bias+cast is split across ACT and DVE (alternating for net o) so
    neither engine bottlenecks the 4x-faster PE.
  - Layer 2 stays bf16 (tiny matmuls are ~free: cost = out rows only).
"""

import os
import sys

for _p in ("/opt/trn_rl_repo", "/root/.axon_site/_ro/trn_rl_repo"):
    if os.path.isdir(_p) and _p not in sys.path:
        sys.path.insert(0, _p)

from contextlib import ExitStack

import ml_dtypes
import numpy as np

import concourse.bass as bass
import concourse.bacc as bacc
import concourse.mybir as mybir
import concourse.tile as tile
from concourse.bass_utils import run_bass_kernel_spmd

B, D_IN, D_H, C = 16384, 1024, 4096, 10
TAU, T1, T2 = 0.5, 3, 3
N_CORES = 8
M = B // N_CORES  # rows per core
M_BLK = 512  # rows per outer block
N_MBLK = M // M_BLK  # 4
MM = M_BLK // 128  # 4 partition-chunks per block
KC = D_IN // 128  # 8 contraction chunks (layer 1)
KP = KC // 2  # 4 DoubleRow k-pairs
DC = D_H // 128  # 32 hidden chunks
LAG = 2  # layer-2 matmuls trail layer-1 by this many dc groups

SX, SW = 16.0, 64.0  # fp8 ship scales; products accumulate at SX*SW = 1024
ALPHA = 0.5  # G-trick shrinkage (= P[relu active])

# Per-net fp8 pass structure, in DoubleRow k-pairs (each pair = 256 of the
# 1024-deep contraction):
#   base x8@w8 is always all 4 pairs; R_PAIRS adds x8@r8 (W-residual) on the
#   first R_PAIRS pairs; XL_PAIRS adds xl8@w8 (x-residual) likewise.
# GCORR: False = none, "full" = 0.5*(xhat@(G-Geff) + xl@G) into z2 via tiny
# matmuls (optimal-shrinkage correction of the relu-masked quantization
# error), "xl" = 0.5*xl@Geff only (when the W-residual is fully compensated).
R_PAIRS = {"o": 0, "f": 1}
XL_PAIRS = {"o": 0, "f": 0}
GCORR = {"o": "full", "f": "full"}

F32 = mybir.dt.float32
BF16 = mybir.dt.bfloat16
F8 = mybir.dt.float8e4
Alu = mybir.AluOpType
Act = mybir.ActivationFunctionType
DR = mybir.MatmulPerfMode.DoubleRow

NEED_XL = any(p > 0 for p in XL_PAIRS.values()) or any(GCORR.values())

LAST_RESULTS = None
_BUILD_CACHE = {}


def _build_module(zero_b2=False):
    nc = bacc.Bacc(
        "TRN2", target_bir_lowering=False, debug=False, num_devices=N_CORES
    )

    xT_d = nc.dram_tensor("xT", [D_IN, M], F8, kind="ExternalInput")
    xlT_d = (
        nc.dram_tensor("xlT", [D_IN, M], F8, kind="ExternalInput")
        if NEED_XL
        else None
    )
    w1_d, r1_d, w2_d, b1_d, b2_d, dm_d, gm_d = {}, {}, {}, {}, {}, {}, {}
    for n in ("o", "f"):
        w1_d[n] = nc.dram_tensor(f"w1{n}", [D_IN, D_H], F8, kind="ExternalInput")
        if R_PAIRS[n]:
            # only the first R_PAIRS*256 contraction rows of the residual ship
            r1_d[n] = nc.dram_tensor(
                f"r1{n}", [R_PAIRS[n] * 256, D_H], F8, kind="ExternalInput"
            )
        # Small tensors ship host-packed as [128, ...] so the DMA is one
        # contiguous run per partition (128 descriptors, not 4096).
        w2_d[n] = nc.dram_tensor(f"w2{n}", [128, DC, C], BF16, kind="ExternalInput")
        b1_d[n] = nc.dram_tensor(f"b1{n}", [128, DC], F32, kind="ExternalInput")
        if not zero_b2:
            b2_d[n] = nc.dram_tensor(f"b2{n}", [128, C], F32, kind="ExternalInput")
        if GCORR[n] == "full":
            dm_d[n] = nc.dram_tensor(f"dm{n}", [128, KC, C], BF16, kind="ExternalInput")
        if GCORR[n]:
            gm_d[n] = nc.dram_tensor(f"gm{n}", [128, KC, C], BF16, kind="ExternalInput")
    out_d = nc.dram_tensor("out", [M, C], F32, kind="ExternalOutput")

    with tile.TileContext(nc) as tc, ExitStack() as ctx:
        consts = ctx.enter_context(tc.tile_pool(name="consts", bufs=1))
        hpool = ctx.enter_context(tc.tile_pool(name="hpool", bufs=8))
        epool = ctx.enter_context(tc.tile_pool(name="epool", bufs=3))
        opool = ctx.enter_context(tc.tile_pool(name="opool", bufs=3))
        psum_h = ctx.enter_context(tc.tile_pool(name="psum_h", bufs=6, space="PSUM"))
        psum_o = ctx.enter_context(tc.tile_pool(name="psum_o", bufs=2, space="PSUM"))

        # ---- resident tensors -------------------------------------------
        GS = 512  # dh elements per weight group tile
        NG = D_H // GS  # 8 groups
        DC_G = GS // 128  # 4 dh chunks per group

        xs_sb = [None] * N_MBLK
        xls_sb = [None] * N_MBLK

        def load_x_blk(blk, which):
            d, store = (xT_d, xs_sb) if which == "x" else (xlT_d, xls_sb)
            t = consts.tile(
                [128, KC, M_BLK], F8, name=f"{which}{blk}", tag=f"{which}{blk}"
            )
            nc.sync.dma_start(
                t[:],
                d.ap()[:, blk * M_BLK : (blk + 1) * M_BLK].rearrange(
                    "(kc p) m -> p kc m", p=128
                ),
            )
            store[blk] = t

        # PE pre-warm: dummy matmuls during the initial weight DMA burn the
        # p-state ramp instead of real work. One small tile doubles as both
        # operands so the first warm matmul issues as early as possible.
        warm_w = consts.tile([128, 128], BF16, name="warm_w", tag="warm_w")
        nc.vector.memset(warm_w[:], 0.0)
        for _ in range(70):
            ph = psum_h.tile([128, M_BLK], F32, name="ph", tag="ph")
            nc.tensor.matmul(ph[:, 0:128], lhsT=warm_w[:], rhs=warm_w[:])

        w1_sb = {n: [] for n in ("o", "f")}
        r1_sb = {n: [] for n in ("o", "f")}

        def load_w_group(d, store, nm, g, nkc=KC):
            t = consts.tile([128, nkc, GS], F8, name=f"{nm}g{g}", tag=f"{nm}g{g}")
            nc.sync.dma_start(
                t[:],
                d.ap()[:, g * GS : (g + 1) * GS].rearrange(
                    "(kc p) d -> p kc d", p=128
                ),
            )
            store.append(t)

        # DMA issue order = on-device consumption order. Each DMA costs
        # ~625ns of HWDGE descriptor-gen regardless of size, so small tensors
        # are spread out to avoid pushing back the w1o group stream that
        # feeds the PE's first microseconds.
        w2_sb, b1_sb, b2_sb, dm_sb, gm_sb = {}, {}, {}, {}, {}
        d_by = {"w2": w2_d, "b1": b1_d, "b2": b2_d, "dm": dm_d, "gm": gm_d}
        sb_by = {"w2": w2_sb, "b1": b1_sb, "b2": b2_sb, "dm": dm_sb, "gm": gm_sb}

        def small(kind, n):
            shape, dt = {
                "w2": ([128, DC, C], BF16),
                "b1": ([128, DC], F32),
                "b2": ([128, C], F32),
                "dm": ([128, KC, C], BF16),
                "gm": ([128, KC, C], BF16),
            }[kind]
            t = consts.tile(shape, dt, name=f"{kind}{n}", tag=f"{kind}{n}")
            nc.sync.dma_start(t[:], d_by[kind][n].ap())
            sb_by[kind][n] = t

        def load_f_group(g):
            load_w_group(w1_d["f"], w1_sb["f"], "w1f", g)
            if R_PAIRS["f"]:
                load_w_group(
                    r1_d["f"], r1_sb["f"], "r1f", g, nkc=2 * R_PAIRS["f"]
                )

        load_w_group(w1_d["o"], w1_sb["o"], "w1o", 0)
        load_x_blk(0, "x")
        small("b1", "o")
        small("w2", "o")
        for g in range(1, 7):
            load_w_group(w1_d["o"], w1_sb["o"], "w1o", g)
        if R_PAIRS["o"]:
            for g in range(NG):
                load_w_group(
                    r1_d["o"], r1_sb["o"], "r1o", g, nkc=2 * R_PAIRS["o"]
                )
        load_f_group(0)
        load_w_group(w1_d["o"], w1_sb["o"], "w1o", 7)
        if NEED_XL:
            load_x_blk(0, "xl")
        if GCORR["o"] == "full":
            small("dm", "o")
        if GCORR["o"]:
            small("gm", "o")
        load_f_group(1)
        small("b1", "f")
        small("w2", "f")
        if not zero_b2:
            small("b2", "o")
            small("b2", "f")
        if GCORR["f"] == "full":
            small("dm", "f")
        if GCORR["f"]:
            small("gm", "f")
        for g in range(2, NG):
            load_f_group(g)
        for blk in range(1, N_MBLK):
            load_x_blk(blk, "x")
            if NEED_XL:
                load_x_blk(blk, "xl")

        # ---- main loop ---------------------------------------------------
        for blk in range(N_MBLK):
            m0 = blk * M_BLK
            probs = {}
            for n in ("o", "f"):
                po = psum_o.tile([128, MM, C], F32, name="po", tag="po")
                # PSUM start_tensor_calc zeroing is region-granular, so four
                # interleaved sub-bank accumulation groups cannot each use
                # start=True: zero the bank once and accumulate throughout.
                nc.vector.memset(po[:], 0.0)
                hT_ring = [None] * DC

                def emit_l2(dc, last_l2):
                    hT = hT_ring[dc]
                    for mm in range(MM):
                        nc.tensor.matmul(
                            po[:, mm, :],
                            lhsT=hT[:, mm * 128 : (mm + 1) * 128],
                            rhs=w2_sb[n][:, dc, :],
                            start=False,
                            stop=last_l2,
                            skip_group_check=True,
                        )

                for dc in range(DC):
                    g, dl = dc // DC_G, dc % DC_G
                    dsl = slice(dl * 128, (dl + 1) * 128)
                    ph = psum_h.tile([128, M_BLK], F32, name="ph", tag="ph")
                    # (weights, moving x, #pairs) for each fp8 pass
                    plist = [(w1_sb[n][g], xs_sb[blk], KP)]
                    if R_PAIRS[n]:
                        plist.append((r1_sb[n][g], xs_sb[blk], R_PAIRS[n]))
                    if XL_PAIRS[n]:
                        plist.append((w1_sb[n][g], xls_sb[blk], XL_PAIRS[n]))
                    np_total = sum(p[2] for p in plist)
                    p_i = 0
                    for wt, xt, npair in plist:
                        for j in range(npair):
                            nc.tensor.matmul(
                                ph[:],
                                lhsT=wt[:, 2 * j : 2 * j + 2, dsl],
                                rhs=xt[:, 2 * j : 2 * j + 2, :],
                                start=(p_i == 0),
                                stop=(p_i == np_total - 1),
                                perf_mode=DR,
                            )
                            p_i += 1
                    hT = hpool.tile([128, M_BLK], BF16, name="hT", tag="hT")
                    # Alternate relu between ACT and DVE: a single engine
                    # (~611ns per tile) cannot keep pace with the fp8 PE.
                    # net f's first few relus stay on ACT so the (in-order)
                    # DVE can drain the o-softmax/tau/base chain first.
                    use_dve = dc % 2 == 1 and (n == "o" or dc >= 7)
                    if use_dve:
                        nc.vector.tensor_scalar(
                            hT[:],
                            ph[:],
                            b1_sb[n][:, dc : dc + 1],
                            0.0,
                            Alu.add,
                            Alu.max,
                        )
                    else:
                        nc.scalar.activation(
                            hT[:], ph[:], Act.Relu, bias=b1_sb[n][:, dc : dc + 1]
                        )
                    hT_ring[dc] = hT
                    if dc >= LAG:
                        emit_l2(dc - LAG, False)

                # G-trick correction matmuls have no relu dependency, so they
                # go before the trailing L2 flush to keep the PE busy while
                # the last relus drain. The flush L2s then carry the group
                # stop flags.
                if GCORR[n]:
                    for mm in range(MM):
                        msl = slice(mm * 128, (mm + 1) * 128)
                        if GCORR[n] == "full":
                            for kc in range(KC):
                                nc.tensor.matmul(
                                    po[:, mm, :],
                                    lhsT=xs_sb[blk][:, kc, msl],
                                    rhs=dm_sb[n][:, kc, :],
                                    start=False,
                                    stop=False,
                                    skip_group_check=True,
                                )
                        for kc in range(KC):
                            nc.tensor.matmul(
                                po[:, mm, :],
                                lhsT=xls_sb[blk][:, kc, msl],
                                rhs=gm_sb[n][:, kc, :],
                                start=False,
                                stop=False,
                                skip_group_check=True,
                            )
                for dc in range(DC - LAG, DC):
                    emit_l2(dc, dc == DC - 1)

                # softmax over C (no max-subtraction: |z| <= ~8 is exp-safe)
                exps = epool.tile([128, MM, C], F32, name=f"ex{n}", tag=f"ex{n}")
                sums = epool.tile([128, MM], F32, name=f"sm{n}", tag=f"sm{n}")
                rinv = epool.tile([128, MM], F32, name=f"ri{n}", tag=f"ri{n}")
                if zero_b2:
                    zin = po[:]
                else:
                    z = epool.tile([128, MM, C], F32, name=f"z{n}", tag=f"z{n}")
                    nc.vector.tensor_tensor(
                        z[:],
                        po[:],
                        b2_sb[n][:, None, :].to_broadcast([128, MM, C]),
                        Alu.add,
                    )
                    zin = z[:]
                nc.scalar.activation(exps[:], zin, Act.Exp)
                nc.vector.tensor_reduce(
                    sums[:], exps[:], axis=mybir.AxisListType.X, op=Alu.add
                )
                nc.vector.reciprocal(rinv[:], sums[:])
                probs[n] = (exps, rinv)

                if n == "o":
                    # pr_o, comp_max_tau, and x1*(1-cond) are all emitted in
                    # the o-section so the DVE chews through them during
                    # net-f's matmuls instead of in the tail.
                    pr = epool.tile([128, MM, C], F32, name="pro", tag="pro")
                    nc.vector.tensor_tensor(
                        pr[:],
                        exps[:],
                        rinv[:, :, None].to_broadcast([128, MM, C]),
                        Alu.mult,
                    )
                    res = epool.tile([128, MM, C + 1], F32, name="res", tag="res")
                    s4 = epool.tile([128, MM], F32, name="s4", tag="s4")
                    u4 = epool.tile([128, MM], F32, name="u4", tag="u4")
                    b4 = epool.tile([128, MM], F32, name="b4", tag="b4")
                    a4 = epool.tile([128, MM], F32, name="a4", tag="a4")
                    nc.vector.tensor_scalar(
                        res[:, :, 0:C], pr[:], 0.0, None, Alu.add
                    )
                    nc.vector.memset(res[:, :, C : C + 1], TAU)
                    for i in range(T1):
                        m_i = 2.0 + TAU * TAU if i == 0 else 2.0
                        k_i = 2.0 / m_i
                        nc.vector.tensor_tensor(res[:], res[:], res[:], Alu.mult)
                        nc.vector.tensor_reduce(
                            s4[:], res[:], axis=mybir.AxisListType.X, op=Alu.add
                        )
                        nc.vector.tensor_scalar(u4[:], s4[:], k_i, None, Alu.mult)
                        nc.vector.tensor_scalar(
                            b4[:], u4[:], -1.0, 1.0, Alu.mult, Alu.add
                        )
                        nc.vector.tensor_scalar(
                            a4[:], u4[:], -k_i, 2.0 * k_i, Alu.mult, Alu.add
                        )
                        for _ in range(T2):
                            nc.vector.tensor_tensor(b4[:], b4[:], b4[:], Alu.mult)
                            nc.vector.scalar_tensor_tensor(
                                a4[:], b4[:], 1.0, a4[:], Alu.add, Alu.mult
                            )
                        nc.vector.tensor_tensor(
                            res[:],
                            res[:],
                            a4[:, :, None].to_broadcast([128, MM, C + 1]),
                            Alu.mult,
                        )
                    # base = x1 * (1 - cond)
                    omc = epool.tile([128, MM], F32, name="omc", tag="omc")
                    nc.vector.tensor_scalar(
                        omc[:], res[:, :, C], -1.0, 1.0, Alu.mult, Alu.add
                    )
                    base = epool.tile([128, MM, C], F32, name="base", tag="base")
                    nc.vector.tensor_tensor(
                        base[:],
                        pr[:],
                        omc[:, :, None].to_broadcast([128, MM, C]),
                        Alu.mult,
                    )

            # ---- blend tail: out = base + cond * exps_f * rinv_f ---------
            exps_f, rinv_f = probs["f"]
            dd = epool.tile([128, MM, C], F32, name="dd", tag="dd")
            outt = opool.tile([128, MM, C], F32, name="outt", tag="outt")
            nc.vector.tensor_tensor(
                dd[:],
                exps_f[:],
                res[:, :, C : C + 1].to_broadcast([128, MM, C]),
                Alu.mult,
            )
            nc.vector.tensor_tensor(
                dd[:],
                dd[:],
                rinv_f[:, :, None].to_broadcast([128, MM, C]),
                Alu.mult,
            )
            nc.vector.tensor_tensor(outt[:], dd[:], base[:], Alu.add)
            nc.sync.dma_start(
                out_d.ap()[m0 : m0 + M_BLK, :].rearrange("(mm p) c -> p mm c", p=128),
                outt[:],
            )

    nc.compile()
    return nc


def _get_module(zero_b2=None):
    if zero_b2 is None:
        # no-arg call (e.g. timing harness): return the last-built module
        if _BUILD_CACHE:
            return _BUILD_CACHE[next(reversed(_BUILD_CACHE))]
        zero_b2 = False
    key = ("nc", zero_b2)
    if key not in _BUILD_CACHE:
        _BUILD_CACHE[key] = _build_module(zero_b2)
    return _BUILD_CACHE[key]


def kernel(x, W1o, b1o, W2o, b2o, W1f, b1f, W2f, b2f):
    f8 = ml_dtypes.float8_e4m3
    bf = ml_dtypes.bfloat16

    x = np.asarray(x, np.float32)
    x16 = SX * x
    x8 = x16.astype(f8)
    xl8 = (x16 - x8.astype(np.float32)).astype(f8) if NEED_XL else None

    W1 = {"o": np.asarray(W1o, np.float32), "f": np.asarray(W1f, np.float32)}
    W2 = {"o": np.asarray(W2o, np.float32), "f": np.asarray(W2f, np.float32)}
    b1 = {"o": np.asarray(b1o, np.float32), "f": np.asarray(b1f, np.float32)}
    b2 = {"o": np.asarray(b2o, np.float32), "f": np.asarray(b2f, np.float32)}

    def pack_p(a):
        # [K*128, ...] -> [128, K, ...]: partition-major so the DMA is one
        # contiguous run per partition.
        a = np.asarray(a)
        out = a.reshape(a.shape[0] // 128, 128, *a.shape[1:]).swapaxes(0, 1)
        return np.ascontiguousarray(out)

    w8, r8, w2s, b1s, b2s, dms, gms = {}, {}, {}, {}, {}, {}, {}
    for n in ("o", "f"):
        w64 = SW * W1[n]
        w8[n] = np.ascontiguousarray(w64.astype(f8))
        r_k = R_PAIRS[n] * 256
        if r_k:
            r8[n] = np.ascontiguousarray(
                (w64[:r_k] - w8[n][:r_k].astype(np.float32)).astype(f8)
            )
        w2s[n] = pack_p((W2[n] / (SX * SW)).astype(bf))
        b1s[n] = pack_p((SX * SW) * b1[n])
        b2s[n] = np.ascontiguousarray(
            np.broadcast_to(b2[n], (128, C)).astype(np.float32)
        )
        if GCORR[n]:
            G = W1[n].astype(np.float64) @ W2[n].astype(np.float64)
            gms[n] = pack_p((ALPHA * G / SX).astype(np.float32).astype(bf))
            if GCORR[n] == "full":
                # effective on-device W1 = w8 plus whatever residual passes run
                weff = w8[n].astype(np.float64)
                if r_k:
                    weff[:r_k] += r8[n].astype(np.float64)
                Geff = (weff / SW) @ W2[n].astype(np.float64)
                dms[n] = pack_p(
                    (ALPHA * (G - Geff) / SX).astype(np.float32).astype(bf)
                )

    zero_b2 = bool(np.all(b2["o"] == 0.0) and np.all(b2["f"] == 0.0))
    nc = _get_module(zero_b2)

    in_maps = []
    for i in range(N_CORES):
        m = {"xT": np.ascontiguousarray(x8[i * M : (i + 1) * M, :].T)}
        if NEED_XL:
            m["xlT"] = np.ascontiguousarray(xl8[i * M : (i + 1) * M, :].T)
        for n in ("o", "f"):
            m[f"w1{n}"] = w8[n]
            if R_PAIRS[n]:
                m[f"r1{n}"] = r8[n]
            m[f"w2{n}"] = w2s[n]
            m[f"b1{n}"] = b1s[n]
            if not zero_b2:
                m[f"b2{n}"] = b2s[n]
            if GCORR[n] == "full":
                m[f"dm{n}"] = dms[n]
            if GCORR[n]:
                m[f"gm{n}"] = gms[n]
        in_maps.append(m)

    trace = bool(os.environ.get("KERNEL_TRACE"))
    results = run_bass_kernel_spmd(
        nc, in_maps, list(range(N_CORES)), trace=trace
    )
    global LAST_RESULTS
    LAST_RESULTS = results

    out = np.concatenate(
        [np.asarray(results.results[i]["out"], np.float32) for i in range(N_CORES)],
        axis=0,
    )
    return out


# revision 53
# speedup vs baseline: 1.0033x; 1.0033x over previous
"""Trainium2 Bass kernel for nn_CombNetHE — fp8 DoubleRow version.

Strategy vs the bf16 baseline:
  - Layer 1 runs in fp8 (e4m3) with MatmulPerfMode.DoubleRow: each matmul
    contracts 2 k-chunks (256 deep) at 0.5 cycles/row -> 4x bf16 throughput.
  - Quantization error is handled per net:
      net o (feeds comp_max_tau + the (1-cond) branch): 1 fp8 pass + the
        "G-trick": z2 += 0.5*(xhat@(G-Ghat) + xl@G) accumulated straight into
        the layer-2 PSUM via tiny [1024,10] matmuls with host-precomputed
        G = W1@W2 matrices. Optimal-shrinkage correction of the relu-masked
        pre-activation error (rho^2 = 1/2 -> sqrt(2) error reduction, ~free).
      net f (dominates the output since cond ~= 1 for most rows): 3 fp8
        passes (x8@w8 + x8@r8 + xl8@w8, residual-compensated to ~bf16
        accuracy).
  - Scale folding: ship 16*x, 64*W1 (and residuals at the same scales) so all
    passes accumulate at scale 1024; b1 is shipped *1024 and W2 /1024, so no
    on-device descaling is needed anywhere.
  - relu+bias+cast is split across ACT and DVE (alternating for net o) so
    neither engine bottlenecks the 4x-faster PE.
  - Layer 2 stays bf16 (tiny matmuls are ~free: cost = out rows only).
"""

import os
import sys

for _p in ("/opt/trn_rl_repo", "/root/.axon_site/_ro/trn_rl_repo"):
    if os.path.isdir(_p) and _p not in sys.path:
        sys.path.insert(0, _p)

from contextlib import ExitStack

import ml_dtypes
import numpy as np

import concourse.bass as bass
import concourse.bacc as bacc
import concourse.mybir as mybir
import concourse.tile as tile
from concourse.bass_utils import run_bass_kernel_spmd

B, D_IN, D_H, C = 16384, 1024, 4096, 10
TAU, T1, T2 = 0.5, 3, 3
N_CORES = 8
M = B // N_CORES  # rows per core
M_BLK = 512  # rows per outer block
N_MBLK = M // M_BLK  # 4
MM = M_BLK // 128  # 4 partition-chunks per block
KC = D_IN // 128  # 8 contraction chunks (layer 1)
KP = KC // 2  # 4 DoubleRow k-pairs
DC = D_H // 128  # 32 hidden chunks
LAG = 2  # layer-2 matmuls trail layer-1 by this many dc groups

SX, SW = 16.0, 64.0  # fp8 ship scales; products accumulate at SX*SW = 1024
ALPHA = 0.5  # G-trick shrinkage (= P[relu active])

# Per-net fp8 pass structure, in DoubleRow k-pairs (each pair = 256 of the
# 1024-deep contraction):
#   base x8@w8 is always all 4 pairs; R_PAIRS adds x8@r8 (W-residual) on the
#   first R_PAIRS pairs; XL_PAIRS adds xl8@w8 (x-residual) likewise.
# GCORR: False = none, "full" = 0.5*(xhat@(G-Geff) + xl@G) into z2 via tiny
# matmuls (optimal-shrinkage correction of the relu-masked quantization
# error), "xl" = 0.5*xl@Geff only (when the W-residual is fully compensated).
R_PAIRS = {"o": 0, "f": 1}
XL_PAIRS = {"o": 0, "f": 0}
GCORR = {"o": "full", "f": "full"}

F32 = mybir.dt.float32
BF16 = mybir.dt.bfloat16
F8 = mybir.dt.float8e4
Alu = mybir.AluOpType
Act = mybir.ActivationFunctionType
DR = mybir.MatmulPerfMode.DoubleRow

NEED_XL = any(p > 0 for p in XL_PAIRS.values()) or any(GCORR.values())

LAST_RESULTS = None
_BUILD_CACHE = {}


def _build_module(zero_b2=False):
    nc = bacc.Bacc(
        "TRN2", target_bir_lowering=False, debug=False, num_devices=N_CORES
    )

    xT_d = nc.dram_tensor("xT", [D_IN, M], F8, kind="ExternalInput")
    xlT_d = (
        nc.dram_tensor("xlT", [D_IN, M], F8, kind="ExternalInput")
        if NEED_XL
        else None
    )
    w1_d, r1_d, w2_d, b1_d, b2_d, dm_d, gm_d = {}, {}, {}, {}, {}, {}, {}
    for n in ("o", "f"):
        w1_d[n] = nc.dram_tensor(f"w1{n}", [D_IN, D_H], F8, kind="ExternalInput")
        if R_PAIRS[n]:
            # only the first R_PAIRS*256 contraction rows of the residual ship
            r1_d[n] = nc.dram_tensor(
                f"r1{n}", [R_PAIRS[n] * 256, D_H], F8, kind="ExternalInput"
            )
        # Small tensors ship host-packed as [128, ...] so the DMA is one
        # contiguous run per partition (128 descriptors, not 4096).
        w2_d[n] = nc.dram_tensor(f"w2{n}", [128, DC, C], BF16, kind="ExternalInput")
        b1_d[n] = nc.dram_tensor(f"b1{n}", [128, DC], F32, kind="ExternalInput")
        if not zero_b2:
            b2_d[n] = nc.dram_tensor(f"b2{n}", [128, C], F32, kind="ExternalInput")
        if GCORR[n] == "full":
            dm_d[n] = nc.dram_tensor(f"dm{n}", [128, KC, C], BF16, kind="ExternalInput")
        if GCORR[n]:
            gm_d[n] = nc.dram_tensor(f"gm{n}", [128, KC, C], BF16, kind="ExternalInput")
    out_d = nc.dram_tensor("out", [M, C], F32, kind="ExternalOutput")

    with tile.TileContext(nc) as tc, ExitStack() as ctx:
        consts = ctx.enter_context(tc.tile_pool(name="consts", bufs=1))
        hpool = ctx.enter_context(tc.tile_pool(name="hpool", bufs=8))
        epool = ctx.enter_context(tc.tile_pool(name="epool", bufs=3))
        opool = ctx.enter_context(tc.tile_pool(name="opool", bufs=3))
        psum_h = ctx.enter_context(tc.tile_pool(name="psum_h", bufs=6, space="PSUM"))
        psum_o = ctx.enter_context(tc.tile_pool(name="psum_o", bufs=2, space="PSUM"))

        # ---- resident tensors -------------------------------------------
        GS = 512  # dh elements per weight group tile
        NG = D_H // GS  # 8 groups
        DC_G = GS // 128  # 4 dh chunks per group

        xs_sb = [None] * N_MBLK
        xls_sb = [None] * N_MBLK

        def load_x_blk(blk, which):
            d, store = (xT_d, xs_sb) if which == "x" else (xlT_d, xls_sb)
            t = consts.tile(
                [128, KC, M_BLK], F8, name=f"{which}{blk}", tag=f"{which}{blk}"
            )
            nc.sync.dma_start(
                t[:],
                d.ap()[:, blk * M_BLK : (blk + 1) * M_BLK].rearrange(
                    "(kc p) m -> p kc m", p=128
                ),
            )
            store[blk] = t

        # PE pre-warm: dummy matmuls during the initial weight DMA burn the
        # p-state ramp instead of real work. One small tile doubles as both
        # operands so the first warm matmul issues as early as possible.
        warm_w = consts.tile([128, 128], BF16, name="warm_w", tag="warm_w")
        nc.vector.memset(warm_w[:], 0.0)
        for _ in range(70):
            ph = psum_h.tile([128, M_BLK], F32, name="ph", tag="ph")
            nc.tensor.matmul(ph[:, 0:128], lhsT=warm_w[:], rhs=warm_w[:])

        w1_sb = {n: [] for n in ("o", "f")}
        r1_sb = {n: [] for n in ("o", "f")}

        def load_w_group(d, store, nm, g, nkc=KC):
            t = consts.tile([128, nkc, GS], F8, name=f"{nm}g{g}", tag=f"{nm}g{g}")
            nc.sync.dma_start(
                t[:],
                d.ap()[:, g * GS : (g + 1) * GS].rearrange(
                    "(kc p) d -> p kc d", p=128
                ),
            )
            store.append(t)

        # DMA issue order = on-device consumption order. Each DMA costs
        # ~625ns of HWDGE descriptor-gen regardless of size, so small tensors
        # are spread out to avoid pushing back the w1o group stream that
        # feeds the PE's first microseconds.
        w2_sb, b1_sb, b2_sb, dm_sb, gm_sb = {}, {}, {}, {}, {}
        d_by = {"w2": w2_d, "b1": b1_d, "b2": b2_d, "dm": dm_d, "gm": gm_d}
        sb_by = {"w2": w2_sb, "b1": b1_sb, "b2": b2_sb, "dm": dm_sb, "gm": gm_sb}

        def small(kind, n):
            shape, dt = {
                "w2": ([128, DC, C], BF16),
                "b1": ([128, DC], F32),
                "b2": ([128, C], F32),
                "dm": ([128, KC, C], BF16),
                "gm": ([128, KC, C], BF16),
            }[kind]
            t = consts.tile(shape, dt, name=f"{kind}{n}", tag=f"{kind}{n}")
            nc.sync.dma_start(t[:], d_by[kind][n].ap())
            sb_by[kind][n] = t

        def load_f_group(g):
            load_w_group(w1_d["f"], w1_sb["f"], "w1f", g)
            if R_PAIRS["f"]:
                load_w_group(
                    r1_d["f"], r1_sb["f"], "r1f", g, nkc=2 * R_PAIRS["f"]
                )

        load_w_group(w1_d["o"], w1_sb["o"], "w1o", 0)
        load_x_blk(0, "x")
        small("b1", "o")
        small("w2", "o")
        for g in range(1, 7):
            load_w_group(w1_d["o"], w1_sb["o"], "w1o", g)
        if R_PAIRS["o"]:
            for g in range(NG):
                load_w_group(
                    r1_d["o"], r1_sb["o"], "r1o", g, nkc=2 * R_PAIRS["o"]
                )
        load_f_group(0)
        load_w_group(w1_d["o"], w1_sb["o"], "w1o", 7)
        if NEED_XL:
            load_x_blk(0, "xl")
        load_f_group(1)
        small("b1", "f")
        small("w2", "f")
        if not zero_b2:
            small("b2", "o")
            small("b2", "f")
        for n in ("o", "f"):
            if GCORR[n] == "full":
                small("dm", n)
            if GCORR[n]:
                small("gm", n)
        for g in range(2, NG):
            load_f_group(g)
        for blk in range(1, N_MBLK):
            load_x_blk(blk, "x")
            if NEED_XL:
                load_x_blk(blk, "xl")

        # ---- main loop ---------------------------------------------------
        for blk in range(N_MBLK):
            m0 = blk * M_BLK
            probs = {}
            for n in ("o", "f"):
                po = psum_o.tile([128, MM, C], F32, name="po", tag="po")
                # PSUM start_tensor_calc zeroing is region-granular, so four
                # interleaved sub-bank accumulation groups cannot each use
                # start=True: zero the bank once and accumulate throughout.
                nc.vector.memset(po[:], 0.0)
                hT_ring = [None] * DC

                def emit_l2(dc, last_l2):
                    hT = hT_ring[dc]
                    for mm in range(MM):
                        nc.tensor.matmul(
                            po[:, mm, :],
                            lhsT=hT[:, mm * 128 : (mm + 1) * 128],
                            rhs=w2_sb[n][:, dc, :],
                            start=False,
                            stop=last_l2,
                            skip_group_check=True,
                        )

                for dc in range(DC):
                    g, dl = dc // DC_G, dc % DC_G
                    dsl = slice(dl * 128, (dl + 1) * 128)
                    ph = psum_h.tile([128, M_BLK], F32, name="ph", tag="ph")
                    # (weights, moving x, #pairs) for each fp8 pass
                    plist = [(w1_sb[n][g], xs_sb[blk], KP)]
                    if R_PAIRS[n]:
                        plist.append((r1_sb[n][g], xs_sb[blk], R_PAIRS[n]))
                    if XL_PAIRS[n]:
                        plist.append((w1_sb[n][g], xls_sb[blk], XL_PAIRS[n]))
                    np_total = sum(p[2] for p in plist)
                    p_i = 0
                    for wt, xt, npair in plist:
                        for j in range(npair):
                            nc.tensor.matmul(
                                ph[:],
                                lhsT=wt[:, 2 * j : 2 * j + 2, dsl],
                                rhs=xt[:, 2 * j : 2 * j + 2, :],
                                start=(p_i == 0),
                                stop=(p_i == np_total - 1),
                                perf_mode=DR,
                            )
                            p_i += 1
                    hT = hpool.tile([128, M_BLK], BF16, name="hT", tag="hT")
                    # Alternate relu between ACT and DVE: a single engine
                    # (~611ns per tile) cannot keep pace with the fp8 PE.
                    # net f's first few relus stay on ACT so the (in-order)
                    # DVE can drain the o-softmax/tau/base chain first.
                    use_dve = dc % 2 == 1 and (n == "o" or dc >= 7)
                    if use_dve:
                        nc.vector.tensor_scalar(
                            hT[:],
                            ph[:],
                            b1_sb[n][:, dc : dc + 1],
                            0.0,
                            Alu.add,
                            Alu.max,
                        )
                    else:
                        nc.scalar.activation(
                            hT[:], ph[:], Act.Relu, bias=b1_sb[n][:, dc : dc + 1]
                        )
                    hT_ring[dc] = hT
                    if dc >= LAG:
                        emit_l2(dc - LAG, False)

                # G-trick correction matmuls have no relu dependency, so they
                # go before the trailing L2 flush to keep the PE busy while
                # the last relus drain. The flush L2s then carry the group
                # stop flags.
                if GCORR[n]:
                    for mm in range(MM):
                        msl = slice(mm * 128, (mm + 1) * 128)
                        if GCORR[n] == "full":
                            for kc in range(KC):
                                nc.tensor.matmul(
                                    po[:, mm, :],
                                    lhsT=xs_sb[blk][:, kc, msl],
                                    rhs=dm_sb[n][:, kc, :],
                                    start=False,
                                    stop=False,
                                    skip_group_check=True,
                                )
                        for kc in range(KC):
                            nc.tensor.matmul(
                                po[:, mm, :],
                                lhsT=xls_sb[blk][:, kc, msl],
                                rhs=gm_sb[n][:, kc, :],
                                start=False,
                                stop=False,
                                skip_group_check=True,
                            )
                for dc in range(DC - LAG, DC):
                    emit_l2(dc, dc == DC - 1)

                # softmax over C (no max-subtraction: |z| <= ~8 is exp-safe)
                exps = epool.tile([128, MM, C], F32, name=f"ex{n}", tag=f"ex{n}")
                sums = epool.tile([128, MM], F32, name=f"sm{n}", tag=f"sm{n}")
                rinv = epool.tile([128, MM], F32, name=f"ri{n}", tag=f"ri{n}")
                if zero_b2:
                    zin = po[:]
                else:
                    z = epool.tile([128, MM, C], F32, name=f"z{n}", tag=f"z{n}")
                    nc.vector.tensor_tensor(
                        z[:],
                        po[:],
                        b2_sb[n][:, None, :].to_broadcast([128, MM, C]),
                        Alu.add,
                    )
                    zin = z[:]
                nc.scalar.activation(exps[:], zin, Act.Exp)
                nc.vector.tensor_reduce(
                    sums[:], exps[:], axis=mybir.AxisListType.X, op=Alu.add
                )
                nc.vector.reciprocal(rinv[:], sums[:])
                probs[n] = (exps, rinv)

                if n == "o":
                    # pr_o, comp_max_tau, and x1*(1-cond) are all emitted in
                    # the o-section so the DVE chews through them during
                    # net-f's matmuls instead of in the tail.
                    pr = epool.tile([128, MM, C], F32, name="pro", tag="pro")
                    nc.vector.tensor_tensor(
                        pr[:],
                        exps[:],
                        rinv[:, :, None].to_broadcast([128, MM, C]),
                        Alu.mult,
                    )
                    res = epool.tile([128, MM, C + 1], F32, name="res", tag="res")
                    s4 = epool.tile([128, MM], F32, name="s4", tag="s4")
                    u4 = epool.tile([128, MM], F32, name="u4", tag="u4")
                    b4 = epool.tile([128, MM], F32, name="b4", tag="b4")
                    a4 = epool.tile([128, MM], F32, name="a4", tag="a4")
                    nc.vector.tensor_scalar(
                        res[:, :, 0:C], pr[:], 0.0, None, Alu.add
                    )
                    nc.vector.memset(res[:, :, C : C + 1], TAU)
                    for i in range(T1):
                        m_i = 2.0 + TAU * TAU if i == 0 else 2.0
                        k_i = 2.0 / m_i
                        nc.vector.tensor_tensor(res[:], res[:], res[:], Alu.mult)
                        nc.vector.tensor_reduce(
                            s4[:], res[:], axis=mybir.AxisListType.X, op=Alu.add
                        )
                        nc.vector.tensor_scalar(u4[:], s4[:], k_i, None, Alu.mult)
                        nc.vector.tensor_scalar(
                            b4[:], u4[:], -1.0, 1.0, Alu.mult, Alu.add
                        )
                        nc.vector.tensor_scalar(
                            a4[:], u4[:], -k_i, 2.0 * k_i, Alu.mult, Alu.add
                        )
                        for _ in range(T2):
                            nc.vector.tensor_tensor(b4[:], b4[:], b4[:], Alu.mult)
                            nc.vector.scalar_tensor_tensor(
                                a4[:], b4[:], 1.0, a4[:], Alu.add, Alu.mult
                            )
                        nc.vector.tensor_tensor(
                            res[:],
                            res[:],
                            a4[:, :, None].to_broadcast([128, MM, C + 1]),
                            Alu.mult,
                        )
                    # base = x1 * (1 - cond)
                    omc = epool.tile([128, MM], F32, name="omc", tag="omc")
                    nc.vector.tensor_scalar(
                        omc[:], res[:, :, C], -1.0, 1.0, Alu.mult, Alu.add
                    )
                    base = epool.tile([128, MM, C], F32, name="base", tag="base")
                    nc.vector.tensor_tensor(
                        base[:],
                        pr[:],
                        omc[:, :, None].to_broadcast([128, MM, C]),
                        Alu.mult,
                    )

            # ---- blend tail: out = base + cond * exps_f * rinv_f ---------
            exps_f, rinv_f = probs["f"]
            dd = epool.tile([128, MM, C], F32, name="dd", tag="dd")
            outt = opool.tile([128, MM, C], F32, name="outt", tag="outt")
            nc.vector.tensor_tensor(
                dd[:],
                exps_f[:],
                res[:, :, C : C + 1].to_broadcast([128, MM, C]),
                Alu.mult,
            )
            nc.vector.tensor_tensor(
                dd[:],
                dd[:],
                rinv_f[:, :, None].to_broadcast([128, MM, C]),
                Alu.mult,
            )
            nc.vector.tensor_tensor(outt[:], dd[:], base[:], Alu.add)
            nc.sync.dma_start(
                out_d.ap()[m0 : m0 + M_BLK, :].rearrange("(mm p) c -> p mm c", p=128),
                outt[:],
            )

    nc.compile()
    return nc


def _get_module(zero_b2=None):
    if zero_b2 is None:
        # no-arg call (e.g. timing harness): return the last-built module
        if _BUILD_CACHE:
            return _BUILD_CACHE[next(reversed(_BUILD_CACHE))]
        zero_b2 = False
    key = ("nc", zero_b2)
    if key not in _BUILD_CACHE:
        _BUILD_CACHE[key] = _build_module(zero_b2)
    return _BUILD_CACHE[key]


def kernel(x, W1o, b1o, W2o, b2o, W1f, b1f, W2f, b2f):
    f8 = ml_dtypes.float8_e4m3
    bf = ml_dtypes.bfloat16

    x = np.asarray(x, np.float32)
    x16 = SX * x
    x8 = x16.astype(f8)
    xl8 = (x16 - x8.astype(np.float32)).astype(f8) if NEED_XL else None

    W1 = {"o": np.asarray(W1o, np.float32), "f": np.asarray(W1f, np.float32)}
    W2 = {"o": np.asarray(W2o, np.float32), "f": np.asarray(W2f, np.float32)}
    b1 = {"o": np.asarray(b1o, np.float32), "f": np.asarray(b1f, np.float32)}
    b2 = {"o": np.asarray(b2o, np.float32), "f": np.asarray(b2f, np.float32)}

    def pack_p(a):
        # [K*128, ...] -> [128, K, ...]: partition-major so the DMA is one
        # contiguous run per partition.
        a = np.asarray(a)
        out = a.reshape(a.shape[0] // 128, 128, *a.shape[1:]).swapaxes(0, 1)
        return np.ascontiguousarray(out)

    w8, r8, w2s, b1s, b2s, dms, gms = {}, {}, {}, {}, {}, {}, {}
    for n in ("o", "f"):
        w64 = SW * W1[n]
        w8[n] = np.ascontiguousarray(w64.astype(f8))
        r_k = R_PAIRS[n] * 256
        if r_k:
            r8[n] = np.ascontiguousarray(
                (w64[:r_k] - w8[n][:r_k].astype(np.float32)).astype(f8)
            )
        w2s[n] = pack_p((W2[n] / (SX * SW)).astype(bf))
        b1s[n] = pack_p((SX * SW) * b1[n])
        b2s[n] = np.ascontiguousarray(
            np.broadcast_to(b2[n], (128, C)).astype(np.float32)
        )
        if GCORR[n]:
            G = W1[n].astype(np.float64) @ W2[n].astype(np.float64)
            gms[n] = pack_p((ALPHA * G / SX).astype(np.float32).astype(bf))
            if GCORR[n] == "full":
                # effective on-device W1 = w8 plus whatever residual passes run
                weff = w8[n].astype(np.float64)
                if r_k:
                    weff[:r_k] += r8[n].astype(np.float64)
                Geff = (weff / SW) @ W2[n].astype(np.float64)
                dms[n] = pack_p(
                    (ALPHA * (G - Geff) / SX).astype(np.float32).astype(bf)
                )

    zero_b2 = bool(np.all(b2["o"] == 0.0) and np.all(b2["f"] == 0.0))
    nc = _get_module(zero_b2)

    in_maps = []
    for i in range(N_CORES):
        m = {"xT": np.ascontiguousarray(x8[i * M : (i + 1) * M, :].T)}
        if NEED_XL:
            m["xlT"] = np.ascontiguousarray(xl8[i * M : (i + 1) * M, :].T)
        for n in ("o", "f"):
            m[f"w1{n}"] = w8[n]
            if R_PAIRS[n]:
                m[f"r1{n}"] = r8[n]
            m[f"w2{n}"] = w2s[n]
            m[f"b1{n}"] = b1s[n]
            if not zero_b2:
                m[f"b2{n}"] = b2s[n]
            if GCORR[n] == "full":
                m[f"dm{n}"] = dms[n]
            if GCORR[n]:
                m[f"gm{n}"] = gms[n]
        in_maps.append(m)

    trace = bool(os.environ.get("KERNEL_TRACE"))
    results = run_bass_kernel_spmd(
        nc, in_maps, list(range(N_CORES)), trace=trace
    )
    global LAST_RESULTS
    LAST_RESULTS = results

    out = np.concatenate(
        [np.asarray(results.results[i]["out"], np.float32) for i in range(N_CORES)],
        axis=0,
    )
    return out


# revision 59
# speedup vs baseline: 1.0122x; 1.0089x over previous
"""Trainium2 Bass kernel for nn_CombNetHE — fp8 DoubleRow version.

Strategy vs the bf16 baseline:
  - Layer 1 runs in fp8 (e4m3) with MatmulPerfMode.DoubleRow: each matmul
    contracts 2 k-chunks (256 deep) at 0.5 cycles/row -> 4x bf16 throughput.
  - Quantization error is handled per net:
      net o (feeds comp_max_tau + the (1-cond) branch): 1 fp8 pass + the
        "G-trick": z2 += 0.5*(xhat@(G-Ghat) + xl@G) accumulated straight into
        the layer-2 PSUM via tiny [1024,10] matmuls with host-precomputed
        G = W1@W2 matrices. Optimal-shrinkage correction of the relu-masked
        pre-activation error (rho^2 = 1/2 -> sqrt(2) error reduction, ~free).
      net f (dominates the output since cond ~= 1 for most rows): 3 fp8
        passes (x8@w8 + x8@r8 + xl8@w8, residual-compensated to ~bf16
        accuracy).
  - Scale folding: ship 16*x, 64*W1 (and residuals at the same scales) so all
    passes accumulate at scale 1024; b1 is shipped *1024 and W2 /1024, so no
    on-device descaling is needed anywhere.
  - relu+bias+cast is split across ACT and DVE (alternating for net o) so
    neither engine bottlenecks the 4x-faster PE.
  - Layer 2 stays bf16 (tiny matmuls are ~free: cost = out rows only).
"""

import os
import sys

for _p in ("/opt/trn_rl_repo", "/root/.axon_site/_ro/trn_rl_repo"):
    if os.path.isdir(_p) and _p not in sys.path:
        sys.path.insert(0, _p)

from contextlib import ExitStack

import ml_dtypes
import numpy as np

import concourse.bass as bass
import concourse.bacc as bacc
import concourse.mybir as mybir
import concourse.tile as tile
from concourse.bass_utils import run_bass_kernel_spmd

B, D_IN, D_H, C = 16384, 1024, 4096, 10
TAU, T1, T2 = 0.5, 3, 3
N_CORES = 8
M = B // N_CORES  # rows per core
M_BLK = 512  # rows per outer block
N_MBLK = M // M_BLK  # 4
MM = M_BLK // 128  # 4 partition-chunks per block
KC = D_IN // 128  # 8 contraction chunks (layer 1)
KP = KC // 2  # 4 DoubleRow k-pairs
DC = D_H // 128  # 32 hidden chunks
LAG = 2  # layer-2 matmuls trail layer-1 by this many dc groups

SX, SW = 16.0, 64.0  # fp8 ship scales; products accumulate at SX*SW = 1024
ALPHA = 0.5  # G-trick shrinkage (= P[relu active])

# Per-net fp8 pass structure, in DoubleRow k-pairs (each pair = 256 of the
# 1024-deep contraction):
#   base x8@w8 is always all 4 pairs; R_PAIRS adds x8@r8 (W-residual) on the
#   first R_PAIRS pairs; XL_PAIRS adds xl8@w8 (x-residual) likewise.
# GCORR: False = none, "full" = 0.5*(xhat@(G-Geff) + xl@G) into z2 via tiny
# matmuls (optimal-shrinkage correction of the relu-masked quantization
# error), "xl" = 0.5*xl@Geff only (when the W-residual is fully compensated).
R_PAIRS = {"o": 0, "f": 1}
XL_PAIRS = {"o": 0, "f": 0}
GCORR = {"o": "full", "f": "full"}

F32 = mybir.dt.float32
BF16 = mybir.dt.bfloat16
F8 = mybir.dt.float8e4
Alu = mybir.AluOpType
Act = mybir.ActivationFunctionType
DR = mybir.MatmulPerfMode.DoubleRow

NEED_XL = any(p > 0 for p in XL_PAIRS.values()) or any(GCORR.values())

LAST_RESULTS = None
_BUILD_CACHE = {}


def _build_module(zero_b2=False):
    nc = bacc.Bacc(
        "TRN2", target_bir_lowering=False, debug=False, num_devices=N_CORES
    )

    xT_d = nc.dram_tensor("xT", [D_IN, M], F8, kind="ExternalInput")
    xlT_d = (
        nc.dram_tensor("xlT", [D_IN, M], F8, kind="ExternalInput")
        if NEED_XL
        else None
    )
    w1_d, r1_d, w2_d, b1_d, b2_d, dm_d, gm_d = {}, {}, {}, {}, {}, {}, {}
    for n in ("o", "f"):
        w1_d[n] = nc.dram_tensor(f"w1{n}", [D_IN, D_H], F8, kind="ExternalInput")
        if R_PAIRS[n]:
            # only the first R_PAIRS*256 contraction rows of the residual ship
            r1_d[n] = nc.dram_tensor(
                f"r1{n}", [R_PAIRS[n] * 256, D_H], F8, kind="ExternalInput"
            )
        # Small tensors ship host-packed as [128, ...] so the DMA is one
        # contiguous run per partition (128 descriptors, not 4096).
        w2_d[n] = nc.dram_tensor(f"w2{n}", [128, DC, C], BF16, kind="ExternalInput")
        b1_d[n] = nc.dram_tensor(f"b1{n}", [128, DC], F32, kind="ExternalInput")
        if not zero_b2:
            b2_d[n] = nc.dram_tensor(f"b2{n}", [128, C], F32, kind="ExternalInput")
    # All G-trick matrices ride in one packed tensor/DMA (each extra DMA
    # costs a ~625ns HWDGE slot that bubbles the engine between small
    # transfers).
    gslots = []
    for n in ("o", "f"):
        if GCORR[n] == "full":
            gslots.append(("dm", n))
        if GCORR[n]:
            gslots.append(("gm", n))
    gidx = {s: i for i, s in enumerate(gslots)}
    gmat_d = (
        nc.dram_tensor("gmat", [128, len(gslots), KC, C], BF16, kind="ExternalInput")
        if gslots
        else None
    )
    out_d = nc.dram_tensor("out", [M, C], F32, kind="ExternalOutput")

    with tile.TileContext(nc) as tc, ExitStack() as ctx:
        consts = ctx.enter_context(tc.tile_pool(name="consts", bufs=1))
        hpool = ctx.enter_context(tc.tile_pool(name="hpool", bufs=8))
        epool = ctx.enter_context(tc.tile_pool(name="epool", bufs=3))
        opool = ctx.enter_context(tc.tile_pool(name="opool", bufs=3))
        psum_h = ctx.enter_context(tc.tile_pool(name="psum_h", bufs=6, space="PSUM"))
        psum_o = ctx.enter_context(tc.tile_pool(name="psum_o", bufs=2, space="PSUM"))

        # ---- resident tensors -------------------------------------------
        GS = 512  # dh elements per weight group tile
        NG = D_H // GS  # 8 groups
        DC_G = GS // 128  # 4 dh chunks per group

        xs_sb = [None] * N_MBLK
        xls_sb = [None] * N_MBLK

        def load_x_blk(blk, which):
            d, store = (xT_d, xs_sb) if which == "x" else (xlT_d, xls_sb)
            t = consts.tile(
                [128, KC, M_BLK], F8, name=f"{which}{blk}", tag=f"{which}{blk}"
            )
            nc.sync.dma_start(
                t[:],
                d.ap()[:, blk * M_BLK : (blk + 1) * M_BLK].rearrange(
                    "(kc p) m -> p kc m", p=128
                ),
            )
            store[blk] = t

        # PE pre-warm: dummy matmuls during the initial weight DMA burn the
        # p-state ramp instead of real work. One small tile doubles as both
        # operands so the first warm matmul issues as early as possible.
        warm_w = consts.tile([128, 128], BF16, name="warm_w", tag="warm_w")
        nc.vector.memset(warm_w[:], 0.0)
        for _ in range(70):
            ph = psum_h.tile([128, M_BLK], F32, name="ph", tag="ph")
            nc.tensor.matmul(ph[:, 0:128], lhsT=warm_w[:], rhs=warm_w[:])

        w1_sb = {n: [] for n in ("o", "f")}
        r1_sb = {n: [] for n in ("o", "f")}

        def load_w_group(d, store, nm, g, nkc=KC):
            t = consts.tile([128, nkc, GS], F8, name=f"{nm}g{g}", tag=f"{nm}g{g}")
            nc.sync.dma_start(
                t[:],
                d.ap()[:, g * GS : (g + 1) * GS].rearrange(
                    "(kc p) d -> p kc d", p=128
                ),
            )
            store.append(t)

        # DMA issue order = on-device consumption order. Each DMA costs
        # ~625ns of HWDGE descriptor-gen regardless of size, so small tensors
        # are spread out to avoid pushing back the w1o group stream that
        # feeds the PE's first microseconds.
        w2_sb, b1_sb, b2_sb = {}, {}, {}
        d_by = {"w2": w2_d, "b1": b1_d, "b2": b2_d}
        sb_by = {"w2": w2_sb, "b1": b1_sb, "b2": b2_sb}

        def small(kind, n):
            shape, dt = {
                "w2": ([128, DC, C], BF16),
                "b1": ([128, DC], F32),
                "b2": ([128, C], F32),
            }[kind]
            t = consts.tile(shape, dt, name=f"{kind}{n}", tag=f"{kind}{n}")
            nc.sync.dma_start(t[:], d_by[kind][n].ap())
            sb_by[kind][n] = t

        gmat_t = None
        if gslots:
            gmat_t = consts.tile(
                [128, len(gslots), KC, C], BF16, name="gmat", tag="gmat"
            )

        def load_f_group(g):
            load_w_group(w1_d["f"], w1_sb["f"], "w1f", g)
            if R_PAIRS["f"]:
                load_w_group(
                    r1_d["f"], r1_sb["f"], "r1f", g, nkc=2 * R_PAIRS["f"]
                )

        load_w_group(w1_d["o"], w1_sb["o"], "w1o", 0)
        load_x_blk(0, "x")
        small("b1", "o")
        small("w2", "o")
        for g in range(1, 7):
            load_w_group(w1_d["o"], w1_sb["o"], "w1o", g)
        if R_PAIRS["o"]:
            for g in range(NG):
                load_w_group(
                    r1_d["o"], r1_sb["o"], "r1o", g, nkc=2 * R_PAIRS["o"]
                )
        load_f_group(0)
        load_w_group(w1_d["o"], w1_sb["o"], "w1o", 7)
        if NEED_XL:
            load_x_blk(0, "xl")
        load_f_group(1)
        small("b1", "f")
        small("w2", "f")
        if not zero_b2:
            small("b2", "o")
            small("b2", "f")
        if gslots:
            nc.sync.dma_start(gmat_t[:], gmat_d.ap())
        for g in range(2, NG):
            load_f_group(g)
        for blk in range(1, N_MBLK):
            load_x_blk(blk, "x")
            if NEED_XL:
                load_x_blk(blk, "xl")

        # ---- main loop ---------------------------------------------------
        for blk in range(N_MBLK):
            m0 = blk * M_BLK
            probs = {}
            for n in ("o", "f"):
                po = psum_o.tile([128, MM, C], F32, name="po", tag="po")
                # PSUM start_tensor_calc zeroing is region-granular, so four
                # interleaved sub-bank accumulation groups cannot each use
                # start=True: zero the bank once and accumulate throughout.
                nc.vector.memset(po[:], 0.0)
                hT_ring = [None] * DC

                def emit_l2(dc, last_l2):
                    hT = hT_ring[dc]
                    for mm in range(MM):
                        nc.tensor.matmul(
                            po[:, mm, :],
                            lhsT=hT[:, mm * 128 : (mm + 1) * 128],
                            rhs=w2_sb[n][:, dc, :],
                            start=False,
                            stop=last_l2,
                            skip_group_check=True,
                        )

                for dc in range(DC):
                    g, dl = dc // DC_G, dc % DC_G
                    dsl = slice(dl * 128, (dl + 1) * 128)
                    ph = psum_h.tile([128, M_BLK], F32, name="ph", tag="ph")
                    # (weights, moving x, #pairs) for each fp8 pass
                    plist = [(w1_sb[n][g], xs_sb[blk], KP)]
                    if R_PAIRS[n]:
                        plist.append((r1_sb[n][g], xs_sb[blk], R_PAIRS[n]))
                    if XL_PAIRS[n]:
                        plist.append((w1_sb[n][g], xls_sb[blk], XL_PAIRS[n]))
                    np_total = sum(p[2] for p in plist)
                    p_i = 0
                    for wt, xt, npair in plist:
                        for j in range(npair):
                            nc.tensor.matmul(
                                ph[:],
                                lhsT=wt[:, 2 * j : 2 * j + 2, dsl],
                                rhs=xt[:, 2 * j : 2 * j + 2, :],
                                start=(p_i == 0),
                                stop=(p_i == np_total - 1),
                                perf_mode=DR,
                            )
                            p_i += 1
                    hT = hpool.tile([128, M_BLK], BF16, name="hT", tag="hT")
                    # Alternate relu between ACT and DVE: a single engine
                    # (~611ns per tile) cannot keep pace with the fp8 PE.
                    # net f's first few relus stay on ACT so the (in-order)
                    # DVE can drain the o-softmax/tau/base chain first.
                    use_dve = dc % 2 == 1 and (n == "o" or dc >= 7)
                    if use_dve:
                        nc.vector.tensor_scalar(
                            hT[:],
                            ph[:],
                            b1_sb[n][:, dc : dc + 1],
                            0.0,
                            Alu.add,
                            Alu.max,
                        )
                    else:
                        nc.scalar.activation(
                            hT[:], ph[:], Act.Relu, bias=b1_sb[n][:, dc : dc + 1]
                        )
                    hT_ring[dc] = hT
                    if dc >= LAG:
                        emit_l2(dc - LAG, False)

                # G-trick correction matmuls have no relu dependency, so they
                # go before the trailing L2 flush to keep the PE busy while
                # the last relus drain. The flush L2s then carry the group
                # stop flags.
                if GCORR[n]:
                    for mm in range(MM):
                        msl = slice(mm * 128, (mm + 1) * 128)
                        if GCORR[n] == "full":
                            for kc in range(KC):
                                nc.tensor.matmul(
                                    po[:, mm, :],
                                    lhsT=xs_sb[blk][:, kc, msl],
                                    rhs=gmat_t[:, gidx[("dm", n)], kc, :],
                                    start=False,
                                    stop=False,
                                    skip_group_check=True,
                                )
                        for kc in range(KC):
                            nc.tensor.matmul(
                                po[:, mm, :],
                                lhsT=xls_sb[blk][:, kc, msl],
                                rhs=gmat_t[:, gidx[("gm", n)], kc, :],
                                start=False,
                                stop=False,
                                skip_group_check=True,
                            )
                for dc in range(DC - LAG, DC):
                    emit_l2(dc, dc == DC - 1)

                # softmax over C (no max-subtraction: |z| <= ~8 is exp-safe)
                exps = epool.tile([128, MM, C], F32, name=f"ex{n}", tag=f"ex{n}")
                sums = epool.tile([128, MM], F32, name=f"sm{n}", tag=f"sm{n}")
                rinv = epool.tile([128, MM], F32, name=f"ri{n}", tag=f"ri{n}")
                if zero_b2:
                    zin = po[:]
                else:
                    z = epool.tile([128, MM, C], F32, name=f"z{n}", tag=f"z{n}")
                    nc.vector.tensor_tensor(
                        z[:],
                        po[:],
                        b2_sb[n][:, None, :].to_broadcast([128, MM, C]),
                        Alu.add,
                    )
                    zin = z[:]
                nc.scalar.activation(exps[:], zin, Act.Exp)
                nc.vector.tensor_reduce(
                    sums[:], exps[:], axis=mybir.AxisListType.X, op=Alu.add
                )
                nc.vector.reciprocal(rinv[:], sums[:])
                probs[n] = (exps, rinv)

                if n == "o":
                    # pr_o, comp_max_tau, and x1*(1-cond) are all emitted in
                    # the o-section so the DVE chews through them during
                    # net-f's matmuls instead of in the tail.
                    pr = epool.tile([128, MM, C], F32, name="pro", tag="pro")
                    nc.vector.tensor_tensor(
                        pr[:],
                        exps[:],
                        rinv[:, :, None].to_broadcast([128, MM, C]),
                        Alu.mult,
                    )
                    res = epool.tile([128, MM, C + 1], F32, name="res", tag="res")
                    s4 = epool.tile([128, MM], F32, name="s4", tag="s4")
                    u4 = epool.tile([128, MM], F32, name="u4", tag="u4")
                    b4 = epool.tile([128, MM], F32, name="b4", tag="b4")
                    a4 = epool.tile([128, MM], F32, name="a4", tag="a4")
                    nc.vector.tensor_scalar(
                        res[:, :, 0:C], pr[:], 0.0, None, Alu.add
                    )
                    nc.vector.memset(res[:, :, C : C + 1], TAU)
                    for i in range(T1):
                        m_i = 2.0 + TAU * TAU if i == 0 else 2.0
                        k_i = 2.0 / m_i
                        nc.vector.tensor_tensor(res[:], res[:], res[:], Alu.mult)
                        nc.vector.tensor_reduce(
                            s4[:], res[:], axis=mybir.AxisListType.X, op=Alu.add
                        )
                        nc.vector.tensor_scalar(u4[:], s4[:], k_i, None, Alu.mult)
                        nc.vector.tensor_scalar(
                            b4[:], u4[:], -1.0, 1.0, Alu.mult, Alu.add
                        )
                        nc.vector.tensor_scalar(
                            a4[:], u4[:], -k_i, 2.0 * k_i, Alu.mult, Alu.add
                        )
                        for _ in range(T2):
                            nc.vector.tensor_tensor(b4[:], b4[:], b4[:], Alu.mult)
                            nc.vector.scalar_tensor_tensor(
                                a4[:], b4[:], 1.0, a4[:], Alu.add, Alu.mult
                            )
                        nc.vector.tensor_tensor(
                            res[:],
                            res[:],
                            a4[:, :, None].to_broadcast([128, MM, C + 1]),
                            Alu.mult,
                        )
                    # base = x1 * (1 - cond)
                    omc = epool.tile([128, MM], F32, name="omc", tag="omc")
                    nc.vector.tensor_scalar(
                        omc[:], res[:, :, C], -1.0, 1.0, Alu.mult, Alu.add
                    )
                    base = epool.tile([128, MM, C], F32, name="base", tag="base")
                    nc.vector.tensor_tensor(
                        base[:],
                        pr[:],
                        omc[:, :, None].to_broadcast([128, MM, C]),
                        Alu.mult,
                    )

            # ---- blend tail: out = base + cond * exps_f * rinv_f ---------
            exps_f, rinv_f = probs["f"]
            dd = epool.tile([128, MM, C], F32, name="dd", tag="dd")
            outt = opool.tile([128, MM, C], F32, name="outt", tag="outt")
            nc.vector.tensor_tensor(
                dd[:],
                exps_f[:],
                res[:, :, C : C + 1].to_broadcast([128, MM, C]),
                Alu.mult,
            )
            nc.vector.tensor_tensor(
                dd[:],
                dd[:],
                rinv_f[:, :, None].to_broadcast([128, MM, C]),
                Alu.mult,
            )
            nc.vector.tensor_tensor(outt[:], dd[:], base[:], Alu.add)
            nc.sync.dma_start(
                out_d.ap()[m0 : m0 + M_BLK, :].rearrange("(mm p) c -> p mm c", p=128),
                outt[:],
            )

    nc.compile()
    return nc


def _get_module(zero_b2=None):
    if zero_b2 is None:
        # no-arg call (e.g. timing harness): return the last-built module
        if _BUILD_CACHE:
            return _BUILD_CACHE[next(reversed(_BUILD_CACHE))]
        zero_b2 = False
    key = ("nc", zero_b2)
    if key not in _BUILD_CACHE:
        _BUILD_CACHE[key] = _build_module(zero_b2)
    return _BUILD_CACHE[key]


def kernel(x, W1o, b1o, W2o, b2o, W1f, b1f, W2f, b2f):
    f8 = ml_dtypes.float8_e4m3
    bf = ml_dtypes.bfloat16

    x = np.asarray(x, np.float32)
    x16 = SX * x
    x8 = x16.astype(f8)
    xl8 = (x16 - x8.astype(np.float32)).astype(f8) if NEED_XL else None

    W1 = {"o": np.asarray(W1o, np.float32), "f": np.asarray(W1f, np.float32)}
    W2 = {"o": np.asarray(W2o, np.float32), "f": np.asarray(W2f, np.float32)}
    b1 = {"o": np.asarray(b1o, np.float32), "f": np.asarray(b1f, np.float32)}
    b2 = {"o": np.asarray(b2o, np.float32), "f": np.asarray(b2f, np.float32)}

    def pack_p(a):
        # [K*128, ...] -> [128, K, ...]: partition-major so the DMA is one
        # contiguous run per partition.
        a = np.asarray(a)
        out = a.reshape(a.shape[0] // 128, 128, *a.shape[1:]).swapaxes(0, 1)
        return np.ascontiguousarray(out)

    w8, r8, w2s, b1s, b2s, dms, gms = {}, {}, {}, {}, {}, {}, {}
    for n in ("o", "f"):
        w64 = SW * W1[n]
        w8[n] = np.ascontiguousarray(w64.astype(f8))
        r_k = R_PAIRS[n] * 256
        if r_k:
            r8[n] = np.ascontiguousarray(
                (w64[:r_k] - w8[n][:r_k].astype(np.float32)).astype(f8)
            )
        w2s[n] = pack_p((W2[n] / (SX * SW)).astype(bf))
        b1s[n] = pack_p((SX * SW) * b1[n])
        b2s[n] = np.ascontiguousarray(
            np.broadcast_to(b2[n], (128, C)).astype(np.float32)
        )
        if GCORR[n]:
            G = W1[n].astype(np.float64) @ W2[n].astype(np.float64)
            gms[n] = pack_p((ALPHA * G / SX).astype(np.float32).astype(bf))
            if GCORR[n] == "full":
                # effective on-device W1 = w8 plus whatever residual passes run
                weff = w8[n].astype(np.float64)
                if r_k:
                    weff[:r_k] += r8[n].astype(np.float64)
                Geff = (weff / SW) @ W2[n].astype(np.float64)
                dms[n] = pack_p(
                    (ALPHA * (G - Geff) / SX).astype(np.float32).astype(bf)
                )

    zero_b2 = bool(np.all(b2["o"] == 0.0) and np.all(b2["f"] == 0.0))
    nc = _get_module(zero_b2)

    # pack all G-trick matrices into one tensor (same slot order as build)
    gstack = []
    for n in ("o", "f"):
        if GCORR[n] == "full":
            gstack.append(dms[n])
        if GCORR[n]:
            gstack.append(gms[n])
    gmat_arr = (
        np.ascontiguousarray(np.stack(gstack, axis=1)) if gstack else None
    )

    in_maps = []
    for i in range(N_CORES):
        m = {"xT": np.ascontiguousarray(x8[i * M : (i + 1) * M, :].T)}
        if NEED_XL:
            m["xlT"] = np.ascontiguousarray(xl8[i * M : (i + 1) * M, :].T)
        if gmat_arr is not None:
            m["gmat"] = gmat_arr
        for n in ("o", "f"):
            m[f"w1{n}"] = w8[n]
            if R_PAIRS[n]:
                m[f"r1{n}"] = r8[n]
            m[f"w2{n}"] = w2s[n]
            m[f"b1{n}"] = b1s[n]
            if not zero_b2:
                m[f"b2{n}"] = b2s[n]
        in_maps.append(m)

    trace = bool(os.environ.get("KERNEL_TRACE"))
    results = run_bass_kernel_spmd(
        nc, in_maps, list(range(N_CORES)), trace=trace
    )
    global LAST_RESULTS
    LAST_RESULTS = results

    out = np.concatenate(
        [np.asarray(results.results[i]["out"], np.float32) for i in range(N_CORES)],
        axis=0,
    )
    return out


# revision 62
# speedup vs baseline: 1.0127x; 1.0005x over previous
"""Trainium2 Bass kernel for nn_CombNetHE — fp8 DoubleRow version.

Strategy vs the bf16 baseline:
  - Layer 1 runs in fp8 (e4m3) with MatmulPerfMode.DoubleRow: each matmul
    contracts 2 k-chunks (256 deep) at 0.5 cycles/row -> 4x bf16 throughput.
  - Quantization error is handled per net:
      net o (feeds comp_max_tau + the (1-cond) branch): 1 fp8 pass + the
        "G-trick": z2 += 0.5*(xhat@(G-Ghat) + xl@G) accumulated straight into
        the layer-2 PSUM via tiny [1024,10] matmuls with host-precomputed
        G = W1@W2 matrices. Optimal-shrinkage correction of the relu-masked
        pre-activation error (rho^2 = 1/2 -> sqrt(2) error reduction, ~free).
      net f (dominates the output since cond ~= 1 for most rows): 3 fp8
        passes (x8@w8 + x8@r8 + xl8@w8, residual-compensated to ~bf16
        accuracy).
  - Scale folding: ship 16*x, 64*W1 (and residuals at the same scales) so all
    passes accumulate at scale 1024; b1 is shipped *1024 and W2 /1024, so no
    on-device descaling is needed anywhere.
  - relu+bias+cast is split across ACT and DVE (alternating for net o) so
    neither engine bottlenecks the 4x-faster PE.
  - Layer 2 stays bf16 (tiny matmuls are ~free: cost = out rows only).
"""

import os
import sys

for _p in ("/opt/trn_rl_repo", "/root/.axon_site/_ro/trn_rl_repo"):
    if os.path.isdir(_p) and _p not in sys.path:
        sys.path.insert(0, _p)

from contextlib import ExitStack

import ml_dtypes
import numpy as np

import concourse.bass as bass
import concourse.bacc as bacc
import concourse.mybir as mybir
import concourse.tile as tile
from concourse.bass_utils import run_bass_kernel_spmd

B, D_IN, D_H, C = 16384, 1024, 4096, 10
TAU, T1, T2 = 0.5, 3, 3
N_CORES = 8
M = B // N_CORES  # rows per core
M_BLK = 512  # rows per outer block
N_MBLK = M // M_BLK  # 4
MM = M_BLK // 128  # 4 partition-chunks per block
KC = D_IN // 128  # 8 contraction chunks (layer 1)
KP = KC // 2  # 4 DoubleRow k-pairs
DC = D_H // 128  # 32 hidden chunks
LAG = 2  # layer-2 matmuls trail layer-1 by this many dc groups

SX, SW = 16.0, 64.0  # fp8 ship scales; products accumulate at SX*SW = 1024
ALPHA = 0.5  # G-trick shrinkage (= P[relu active])

# Per-net fp8 pass structure, in DoubleRow k-pairs (each pair = 256 of the
# 1024-deep contraction):
#   base x8@w8 is always all 4 pairs; R_PAIRS adds x8@r8 (W-residual) on the
#   first R_PAIRS pairs; XL_PAIRS adds xl8@w8 (x-residual) likewise.
# GCORR: False = none, "full" = 0.5*(xhat@(G-Geff) + xl@G) into z2 via tiny
# matmuls (optimal-shrinkage correction of the relu-masked quantization
# error), "xl" = 0.5*xl@Geff only (when the W-residual is fully compensated).
R_PAIRS = {"o": 0, "f": 1}
XL_PAIRS = {"o": 0, "f": 0}
GCORR = {"o": "full", "f": "full"}

F32 = mybir.dt.float32
BF16 = mybir.dt.bfloat16
F8 = mybir.dt.float8e4
Alu = mybir.AluOpType
Act = mybir.ActivationFunctionType
DR = mybir.MatmulPerfMode.DoubleRow

NEED_XL = any(p > 0 for p in XL_PAIRS.values()) or any(GCORR.values())

LAST_RESULTS = None
_BUILD_CACHE = {}


def _build_module(zero_b2=False):
    nc = bacc.Bacc(
        "TRN2", target_bir_lowering=False, debug=False, num_devices=N_CORES
    )

    xT_d = nc.dram_tensor("xT", [D_IN, M], F8, kind="ExternalInput")
    xlT_d = (
        nc.dram_tensor("xlT", [D_IN, M], F8, kind="ExternalInput")
        if NEED_XL
        else None
    )
    w1_d, r1_d, w2_d, b1_d, b2_d, dm_d, gm_d = {}, {}, {}, {}, {}, {}, {}
    for n in ("o", "f"):
        w1_d[n] = nc.dram_tensor(f"w1{n}", [D_IN, D_H], F8, kind="ExternalInput")
        if R_PAIRS[n]:
            # only the first R_PAIRS*256 contraction rows of the residual ship
            r1_d[n] = nc.dram_tensor(
                f"r1{n}", [R_PAIRS[n] * 256, D_H], F8, kind="ExternalInput"
            )
        # Small tensors ship host-packed as [128, ...] so the DMA is one
        # contiguous run per partition (128 descriptors, not 4096).
        w2_d[n] = nc.dram_tensor(f"w2{n}", [128, DC, C], BF16, kind="ExternalInput")
        b1_d[n] = nc.dram_tensor(f"b1{n}", [128, DC], F32, kind="ExternalInput")
        if not zero_b2:
            b2_d[n] = nc.dram_tensor(f"b2{n}", [128, C], F32, kind="ExternalInput")
    # All G-trick matrices ride in one packed tensor/DMA (each extra DMA
    # costs a ~625ns HWDGE slot that bubbles the engine between small
    # transfers).
    gslots = []
    for n in ("o", "f"):
        if GCORR[n] == "full":
            gslots.append(("dm", n))
        if GCORR[n]:
            gslots.append(("gm", n))
    gidx = {s: i for i, s in enumerate(gslots)}
    gmat_d = (
        nc.dram_tensor("gmat", [128, len(gslots), KC, C], BF16, kind="ExternalInput")
        if gslots
        else None
    )
    out_d = nc.dram_tensor("out", [M, C], F32, kind="ExternalOutput")

    with tile.TileContext(nc) as tc, ExitStack() as ctx:
        consts = ctx.enter_context(tc.tile_pool(name="consts", bufs=1))
        hpool = ctx.enter_context(tc.tile_pool(name="hpool", bufs=8))
        epool = ctx.enter_context(tc.tile_pool(name="epool", bufs=3))
        opool = ctx.enter_context(tc.tile_pool(name="opool", bufs=3))
        psum_h = ctx.enter_context(tc.tile_pool(name="psum_h", bufs=7, space="PSUM"))
        psum_o = ctx.enter_context(tc.tile_pool(name="psum_o", bufs=1, space="PSUM"))

        # ---- resident tensors -------------------------------------------
        GS = 512  # dh elements per weight group tile
        NG = D_H // GS  # 8 groups
        DC_G = GS // 128  # 4 dh chunks per group

        xs_sb = [None] * N_MBLK
        xls_sb = [None] * N_MBLK

        def load_x_blk(blk, which):
            d, store = (xT_d, xs_sb) if which == "x" else (xlT_d, xls_sb)
            t = consts.tile(
                [128, KC, M_BLK], F8, name=f"{which}{blk}", tag=f"{which}{blk}"
            )
            nc.sync.dma_start(
                t[:],
                d.ap()[:, blk * M_BLK : (blk + 1) * M_BLK].rearrange(
                    "(kc p) m -> p kc m", p=128
                ),
            )
            store[blk] = t

        # PE pre-warm: dummy matmuls during the initial weight DMA burn the
        # p-state ramp instead of real work. One small tile doubles as both
        # operands so the first warm matmul issues as early as possible.
        warm_w = consts.tile([128, 128], BF16, name="warm_w", tag="warm_w")
        nc.vector.memset(warm_w[:], 0.0)
        for _ in range(70):
            ph = psum_h.tile([128, M_BLK], F32, name="ph", tag="ph")
            nc.tensor.matmul(ph[:, 0:128], lhsT=warm_w[:], rhs=warm_w[:])

        w1_sb = {n: [] for n in ("o", "f")}
        r1_sb = {n: [] for n in ("o", "f")}

        def load_w_group(d, store, nm, g, nkc=KC):
            t = consts.tile([128, nkc, GS], F8, name=f"{nm}g{g}", tag=f"{nm}g{g}")
            nc.sync.dma_start(
                t[:],
                d.ap()[:, g * GS : (g + 1) * GS].rearrange(
                    "(kc p) d -> p kc d", p=128
                ),
            )
            store.append(t)

        # DMA issue order = on-device consumption order. Each DMA costs
        # ~625ns of HWDGE descriptor-gen regardless of size, so small tensors
        # are spread out to avoid pushing back the w1o group stream that
        # feeds the PE's first microseconds.
        w2_sb, b1_sb, b2_sb = {}, {}, {}
        d_by = {"w2": w2_d, "b1": b1_d, "b2": b2_d}
        sb_by = {"w2": w2_sb, "b1": b1_sb, "b2": b2_sb}

        def small(kind, n):
            shape, dt = {
                "w2": ([128, DC, C], BF16),
                "b1": ([128, DC], F32),
                "b2": ([128, C], F32),
            }[kind]
            t = consts.tile(shape, dt, name=f"{kind}{n}", tag=f"{kind}{n}")
            nc.sync.dma_start(t[:], d_by[kind][n].ap())
            sb_by[kind][n] = t

        gmat_t = None
        if gslots:
            gmat_t = consts.tile(
                [128, len(gslots), KC, C], BF16, name="gmat", tag="gmat"
            )

        def load_f_group(g):
            load_w_group(w1_d["f"], w1_sb["f"], "w1f", g)
            if R_PAIRS["f"]:
                load_w_group(
                    r1_d["f"], r1_sb["f"], "r1f", g, nkc=2 * R_PAIRS["f"]
                )

        load_w_group(w1_d["o"], w1_sb["o"], "w1o", 0)
        load_x_blk(0, "x")
        small("b1", "o")
        small("w2", "o")
        for g in range(1, 7):
            load_w_group(w1_d["o"], w1_sb["o"], "w1o", g)
        if R_PAIRS["o"]:
            for g in range(NG):
                load_w_group(
                    r1_d["o"], r1_sb["o"], "r1o", g, nkc=2 * R_PAIRS["o"]
                )
        load_f_group(0)
        load_w_group(w1_d["o"], w1_sb["o"], "w1o", 7)
        if NEED_XL:
            load_x_blk(0, "xl")
        load_f_group(1)
        small("b1", "f")
        small("w2", "f")
        if not zero_b2:
            small("b2", "o")
            small("b2", "f")
        if gslots:
            nc.sync.dma_start(gmat_t[:], gmat_d.ap())
        for g in range(2, NG):
            load_f_group(g)
        for blk in range(1, N_MBLK):
            load_x_blk(blk, "x")
            if NEED_XL:
                load_x_blk(blk, "xl")

        # Both nets' layer-2 accumulators share one PSUM bank (two halves of
        # a single tile) so psum_h can hold a 7th bank.
        po2 = psum_o.tile([128, 2, MM, C], F32, name="po2", tag="po2")

        # ---- main loop ---------------------------------------------------
        for blk in range(N_MBLK):
            m0 = blk * M_BLK
            probs = {}
            for n in ("o", "f"):
                po = po2[:, 0 if n == "o" else 1]
                # PSUM start_tensor_calc zeroing is region-granular, so four
                # interleaved sub-bank accumulation groups cannot each use
                # start=True: zero the bank once and accumulate throughout.
                nc.vector.memset(po[:], 0.0)
                hT_ring = [None] * DC

                def emit_l2(dc, last_l2):
                    hT = hT_ring[dc]
                    for mm in range(MM):
                        nc.tensor.matmul(
                            po[:, mm, :],
                            lhsT=hT[:, mm * 128 : (mm + 1) * 128],
                            rhs=w2_sb[n][:, dc, :],
                            start=False,
                            stop=last_l2,
                            skip_group_check=True,
                        )

                for dc in range(DC):
                    g, dl = dc // DC_G, dc % DC_G
                    dsl = slice(dl * 128, (dl + 1) * 128)
                    ph = psum_h.tile([128, M_BLK], F32, name="ph", tag="ph")
                    # (weights, moving x, #pairs) for each fp8 pass
                    plist = [(w1_sb[n][g], xs_sb[blk], KP)]
                    if R_PAIRS[n]:
                        plist.append((r1_sb[n][g], xs_sb[blk], R_PAIRS[n]))
                    if XL_PAIRS[n]:
                        plist.append((w1_sb[n][g], xls_sb[blk], XL_PAIRS[n]))
                    np_total = sum(p[2] for p in plist)
                    p_i = 0
                    for wt, xt, npair in plist:
                        for j in range(npair):
                            nc.tensor.matmul(
                                ph[:],
                                lhsT=wt[:, 2 * j : 2 * j + 2, dsl],
                                rhs=xt[:, 2 * j : 2 * j + 2, :],
                                start=(p_i == 0),
                                stop=(p_i == np_total - 1),
                                perf_mode=DR,
                            )
                            p_i += 1
                    hT = hpool.tile([128, M_BLK], BF16, name="hT", tag="hT")
                    # Alternate relu between ACT and DVE: a single engine
                    # (~611ns per tile) cannot keep pace with the fp8 PE.
                    # net f's first few relus stay on ACT so the (in-order)
                    # DVE can drain the o-softmax/tau/base chain first.
                    use_dve = dc % 2 == 1 and (n == "o" or dc >= 7)
                    if use_dve:
                        nc.vector.tensor_scalar(
                            hT[:],
                            ph[:],
                            b1_sb[n][:, dc : dc + 1],
                            0.0,
                            Alu.add,
                            Alu.max,
                        )
                    else:
                        nc.scalar.activation(
                            hT[:], ph[:], Act.Relu, bias=b1_sb[n][:, dc : dc + 1]
                        )
                    hT_ring[dc] = hT
                    if dc >= LAG:
                        emit_l2(dc - LAG, False)

                # G-trick correction matmuls have no relu dependency, so they
                # go before the trailing L2 flush to keep the PE busy while
                # the last relus drain. The flush L2s then carry the group
                # stop flags.
                if GCORR[n]:
                    for mm in range(MM):
                        msl = slice(mm * 128, (mm + 1) * 128)
                        if GCORR[n] == "full":
                            for kc in range(KC):
                                nc.tensor.matmul(
                                    po[:, mm, :],
                                    lhsT=xs_sb[blk][:, kc, msl],
                                    rhs=gmat_t[:, gidx[("dm", n)], kc, :],
                                    start=False,
                                    stop=False,
                                    skip_group_check=True,
                                )
                        for kc in range(KC):
                            nc.tensor.matmul(
                                po[:, mm, :],
                                lhsT=xls_sb[blk][:, kc, msl],
                                rhs=gmat_t[:, gidx[("gm", n)], kc, :],
                                start=False,
                                stop=False,
                                skip_group_check=True,
                            )
                for dc in range(DC - LAG, DC):
                    emit_l2(dc, dc == DC - 1)

                # softmax over C (no max-subtraction: |z| <= ~8 is exp-safe)
                exps = epool.tile([128, MM, C], F32, name=f"ex{n}", tag=f"ex{n}")
                sums = epool.tile([128, MM], F32, name=f"sm{n}", tag=f"sm{n}")
                rinv = epool.tile([128, MM], F32, name=f"ri{n}", tag=f"ri{n}")
                if zero_b2:
                    zin = po[:]
                else:
                    z = epool.tile([128, MM, C], F32, name=f"z{n}", tag=f"z{n}")
                    nc.vector.tensor_tensor(
                        z[:],
                        po[:],
                        b2_sb[n][:, None, :].to_broadcast([128, MM, C]),
                        Alu.add,
                    )
                    zin = z[:]
                nc.scalar.activation(exps[:], zin, Act.Exp)
                nc.vector.tensor_reduce(
                    sums[:], exps[:], axis=mybir.AxisListType.X, op=Alu.add
                )
                nc.vector.reciprocal(rinv[:], sums[:])
                probs[n] = (exps, rinv)

                if n == "o":
                    # pr_o, comp_max_tau, and x1*(1-cond) are all emitted in
                    # the o-section so the DVE chews through them during
                    # net-f's matmuls instead of in the tail.
                    pr = epool.tile([128, MM, C], F32, name="pro", tag="pro")
                    nc.vector.tensor_tensor(
                        pr[:],
                        exps[:],
                        rinv[:, :, None].to_broadcast([128, MM, C]),
                        Alu.mult,
                    )
                    res = epool.tile([128, MM, C + 1], F32, name="res", tag="res")
                    s4 = epool.tile([128, MM], F32, name="s4", tag="s4")
                    u4 = epool.tile([128, MM], F32, name="u4", tag="u4")
                    b4 = epool.tile([128, MM], F32, name="b4", tag="b4")
                    a4 = epool.tile([128, MM], F32, name="a4", tag="a4")
                    nc.vector.tensor_scalar(
                        res[:, :, 0:C], pr[:], 0.0, None, Alu.add
                    )
                    nc.vector.memset(res[:, :, C : C + 1], TAU)
                    for i in range(T1):
                        m_i = 2.0 + TAU * TAU if i == 0 else 2.0
                        k_i = 2.0 / m_i
                        nc.vector.tensor_tensor(res[:], res[:], res[:], Alu.mult)
                        nc.vector.tensor_reduce(
                            s4[:], res[:], axis=mybir.AxisListType.X, op=Alu.add
                        )
                        nc.vector.tensor_scalar(u4[:], s4[:], k_i, None, Alu.mult)
                        nc.vector.tensor_scalar(
                            b4[:], u4[:], -1.0, 1.0, Alu.mult, Alu.add
                        )
                        nc.vector.tensor_scalar(
                            a4[:], u4[:], -k_i, 2.0 * k_i, Alu.mult, Alu.add
                        )
                        for _ in range(T2):
                            nc.vector.tensor_tensor(b4[:], b4[:], b4[:], Alu.mult)
                            nc.vector.scalar_tensor_tensor(
                                a4[:], b4[:], 1.0, a4[:], Alu.add, Alu.mult
                            )
                        nc.vector.tensor_tensor(
                            res[:],
                            res[:],
                            a4[:, :, None].to_broadcast([128, MM, C + 1]),
                            Alu.mult,
                        )
                    # base = x1 * (1 - cond)
                    omc = epool.tile([128, MM], F32, name="omc", tag="omc")
                    nc.vector.tensor_scalar(
                        omc[:], res[:, :, C], -1.0, 1.0, Alu.mult, Alu.add
                    )
                    base = epool.tile([128, MM, C], F32, name="base", tag="base")
                    nc.vector.tensor_tensor(
                        base[:],
                        pr[:],
                        omc[:, :, None].to_broadcast([128, MM, C]),
                        Alu.mult,
                    )

            # ---- blend tail: out = base + cond * exps_f * rinv_f ---------
            exps_f, rinv_f = probs["f"]
            dd = epool.tile([128, MM, C], F32, name="dd", tag="dd")
            outt = opool.tile([128, MM, C], F32, name="outt", tag="outt")
            nc.vector.tensor_tensor(
                dd[:],
                exps_f[:],
                res[:, :, C : C + 1].to_broadcast([128, MM, C]),
                Alu.mult,
            )
            nc.vector.tensor_tensor(
                dd[:],
                dd[:],
                rinv_f[:, :, None].to_broadcast([128, MM, C]),
                Alu.mult,
            )
            nc.vector.tensor_tensor(outt[:], dd[:], base[:], Alu.add)
            nc.sync.dma_start(
                out_d.ap()[m0 : m0 + M_BLK, :].rearrange("(mm p) c -> p mm c", p=128),
                outt[:],
            )

    nc.compile()
    return nc


def _get_module(zero_b2=None):
    if zero_b2 is None:
        # no-arg call (e.g. timing harness): return the last-built module
        if _BUILD_CACHE:
            return _BUILD_CACHE[next(reversed(_BUILD_CACHE))]
        zero_b2 = False
    key = ("nc", zero_b2)
    if key not in _BUILD_CACHE:
        _BUILD_CACHE[key] = _build_module(zero_b2)
    return _BUILD_CACHE[key]


def kernel(x, W1o, b1o, W2o, b2o, W1f, b1f, W2f, b2f):
    f8 = ml_dtypes.float8_e4m3
    bf = ml_dtypes.bfloat16

    x = np.asarray(x, np.float32)
    x16 = SX * x
    x8 = x16.astype(f8)
    xl8 = (x16 - x8.astype(np.float32)).astype(f8) if NEED_XL else None

    W1 = {"o": np.asarray(W1o, np.float32), "f": np.asarray(W1f, np.float32)}
    W2 = {"o": np.asarray(W2o, np.float32), "f": np.asarray(W2f, np.float32)}
    b1 = {"o": np.asarray(b1o, np.float32), "f": np.asarray(b1f, np.float32)}
    b2 = {"o": np.asarray(b2o, np.float32), "f": np.asarray(b2f, np.float32)}

    def pack_p(a):
        # [K*128, ...] -> [128, K, ...]: partition-major so the DMA is one
        # contiguous run per partition.
        a = np.asarray(a)
        out = a.reshape(a.shape[0] // 128, 128, *a.shape[1:]).swapaxes(0, 1)
        return np.ascontiguousarray(out)

    w8, r8, w2s, b1s, b2s, dms, gms = {}, {}, {}, {}, {}, {}, {}
    for n in ("o", "f"):
        w64 = SW * W1[n]
        w8[n] = np.ascontiguousarray(w64.astype(f8))
        r_k = R_PAIRS[n] * 256
        if r_k:
            r8[n] = np.ascontiguousarray(
                (w64[:r_k] - w8[n][:r_k].astype(np.float32)).astype(f8)
            )
        w2s[n] = pack_p((W2[n] / (SX * SW)).astype(bf))
        b1s[n] = pack_p((SX * SW) * b1[n])
        b2s[n] = np.ascontiguousarray(
            np.broadcast_to(b2[n], (128, C)).astype(np.float32)
        )
        if GCORR[n]:
            G = W1[n].astype(np.float64) @ W2[n].astype(np.float64)
            gms[n] = pack_p((ALPHA * G / SX).astype(np.float32).astype(bf))
            if GCORR[n] == "full":
                # effective on-device W1 = w8 plus whatever residual passes run
                weff = w8[n].astype(np.float64)
                if r_k:
                    weff[:r_k] += r8[n].astype(np.float64)
                Geff = (weff / SW) @ W2[n].astype(np.float64)
                dms[n] = pack_p(
                    (ALPHA * (G - Geff) / SX).astype(np.float32).astype(bf)
                )

    zero_b2 = bool(np.all(b2["o"] == 0.0) and np.all(b2["f"] == 0.0))
    nc = _get_module(zero_b2)

    # pack all G-trick matrices into one tensor (same slot order as build)
    gstack = []
    for n in ("o", "f"):
        if GCORR[n] == "full":
            gstack.append(dms[n])
        if GCORR[n]:
            gstack.append(gms[n])
    gmat_arr = (
        np.ascontiguousarray(np.stack(gstack, axis=1)) if gstack else None
    )

    in_maps = []
    for i in range(N_CORES):
        m = {"xT": np.ascontiguousarray(x8[i * M : (i + 1) * M, :].T)}
        if NEED_XL:
            m["xlT"] = np.ascontiguousarray(xl8[i * M : (i + 1) * M, :].T)
        if gmat_arr is not None:
            m["gmat"] = gmat_arr
        for n in ("o", "f"):
            m[f"w1{n}"] = w8[n]
            if R_PAIRS[n]:
                m[f"r1{n}"] = r8[n]
            m[f"w2{n}"] = w2s[n]
            m[f"b1{n}"] = b1s[n]
            if not zero_b2:
                m[f"b2{n}"] = b2s[n]
        in_maps.append(m)

    trace = bool(os.environ.get("KERNEL_TRACE"))
    results = run_bass_kernel_spmd(
        nc, in_maps, list(range(N_CORES)), trace=trace
    )
    global LAST_RESULTS
    LAST_RESULTS = results

    out = np.concatenate(
        [np.asarray(results.results[i]["out"], np.float32) for i in range(N_CORES)],
        axis=0,
    )
    return out
